# revision 1
# baseline (speedup 1.0000x reference)
"""DARNN (dual-attention RNN) Trainium2 Bass kernel.

Strategy (pure data parallel, 8 cores, B=1024 -> 128 samples/core):

All activations are kept feature-major on-chip: a tensor x[b, f] lives in
SBUF as xT[f, b] with features on partitions and the 128 local batch
elements on the free dim. Every matmul then contracts over the partition
dim with the (pre-transposed, bf16) weight as the stationary operand and
the activation as the moving operand (N = 128).

Key algebraic restructuring: the decoder attention output feat_i is only
ever consumed through linear projections (dec_Wc row and dec_Wf row), so
it is never materialized. Instead q_c[t',b] = hs[b,t',:] @ Wc[0,1:] and
q_f[t',b] = hs[b,t',:] @ Wf[0,HD:] are accumulated during the encoder
(one tiny matmul + DMA per step), and each decoder step only needs
  dot_c[b] = sum_t' e[t',b] q_c[t',b] / Z[b]
computed with an elementwise multiply and a ones-vector matmul reduction
over partitions. The softmax max-subtraction is skipped (scores are
O(1), exp is safe).

Encoder LSTM hidden states hs are stored feature-major in SBUF (bf16) and
reused as decoder attention values. feat is pre-transposed and bf16-cast
on the host and streamed per-step; all weights are packed on the host
into their exact SBUF layouts.
"""

import numpy as np
import ml_dtypes

B, T, NF, HE, HD = 1024, 256, 128, 256, 256
NCORES = 8
BL = B // NCORES  # 128 local batch
TP = T + 1  # 257
BF16 = ml_dtypes.bfloat16

_CACHE = {}


def _bf(x):
    return np.ascontiguousarray(x.astype(BF16))


def _f32(x):
    return np.ascontiguousarray(x.astype(np.float32))


def _pack_inputs(inputs):
    """Pack weights/biases into per-SBUF-tile layouts (shared across cores)."""
    f = {k: np.asarray(v, dtype=np.float32) for k, v in inputs.items()}
    p = {}

    # --- encoder attn1: a = tanh(cat[x,h,c] @ Wa1.T + ba1) ---
    # lhsT tile [128, 5, 257]: [p, k, m] = Wa1[m, k*128+p]
    wa1 = f["enc_Wa1"]  # [257, 640]
    p["w_enc_a1"] = _bf(wa1.T.reshape(5, 128, TP).transpose(1, 0, 2))
    b1 = np.zeros((128, 3), np.float32)
    b1.T.flat[:TP] = f["enc_ba1"]  # [p, j] = ba1[j*128+p]
    p["b_enc_a1"] = _f32(b1)

    # --- encoder attn2: alpha = a @ Wa2.T + ba2 ---  K=257 (3 chunks), M=128
    wa2 = f["enc_Wa2"]  # [128, 257]
    w = np.zeros((128, 3, 128), np.float32)
    w2 = wa2.T  # [257, 128]
    w[:, 0, :] = w2[0:128]
    w[:, 1, :] = w2[128:256]
    w[0, 2, :] = w2[256]
    p["w_enc_a2"] = _bf(w)
    p["b_enc_a2"] = _f32(f["enc_ba2"].reshape(128, 1))

    # --- encoder gates: g = Wih@xi + Whh@h + b ---  K chunks: [xi, h0, h1]
    w = np.zeros((128, 3, 4 * HE), np.float32)
    w[:, 0, :] = f["enc_Wih"].T  # [128, 1024]
    w[:, 1, :] = f["enc_Whh"].T[0:128]
    w[:, 2, :] = f["enc_Whh"].T[128:256]
    p["w_enc_g"] = _bf(w)
    bsum = f["enc_bih"] + f["enc_bhh"]
    p["b_enc_g"] = _f32(bsum.reshape(8, 128).T)  # [p, j] = b[j*128+p]

    # --- q projections: q_c = h . Wc[0,1:], q_f = h . Wf[0,HD:] ---
    w = np.zeros((128, 2, 2), np.float32)
    w[:, 0, 0] = f["dec_Wc"][0, 1 : 1 + 128]
    w[:, 1, 0] = f["dec_Wc"][0, 129 : 1 + 256]
    w[:, 0, 1] = f["dec_Wf"][0, HD : HD + 128]
    w[:, 1, 1] = f["dec_Wf"][0, HD + 128 : HD + 256]
    p["w_q"] = _bf(w)

    # --- decoder attn1: a = tanh(cat[h,c,feat] @ Wa1.T + ba1) --- K=768 (6)
    wa1d = f["dec_Wa1"]  # [256, 768]
    p["w_dec_a1"] = _bf(wa1d.T.reshape(6, 128, HE).transpose(1, 0, 2))
    p["b_dec_a1"] = _f32(f["dec_ba1"].reshape(2, 128).T)

    # --- decoder attn2: s = a @ Wa2.T + ba2 --- K=256 (2), M=257
    wa2d = f["dec_Wa2"]  # [257, 256]
    p["w_dec_a2"] = _bf(wa2d.T.reshape(2, 128, TP).transpose(1, 0, 2))
    b2 = np.zeros((128, 3), np.float32)
    b2.T.flat[:TP] = f["dec_ba2"]
    p["b_dec_a2"] = _f32(b2)

    # --- decoder gates --- K chunks: [xi(K=1), h0, h1]
    w = np.zeros((128, 3, 4 * HD), np.float32)
    w[0, 0, :] = f["dec_Wih"][:, 0]
    w[:, 1, :] = f["dec_Whh"].T[0:128]
    w[:, 2, :] = f["dec_Whh"].T[128:256]
    p["w_dec_g"] = _bf(w)
    bsumd = f["dec_bih"] + f["dec_bhh"]
    p["b_dec_g"] = _f32(bsumd.reshape(8, 128).T)

    # --- final: out = hd . Wf[0,:HD] + dot_f/Z + bf ---
    w = np.zeros((128, 2, 1), np.float32)
    w[:, 0, 0] = f["dec_Wf"][0, 0:128]
    w[:, 1, 0] = f["dec_Wf"][0, 128:256]
    p["w_fh"] = _bf(w)

    # --- scalars: [bc, bf, Wc00, 0] ---
    p["consts"] = _f32(
        np.array([[f["dec_bc"][0], f["dec_bf"][0], f["dec_Wc"][0, 0], 0.0]])
    )

    # --- per-core tensors ---
    feat = f["feat"]  # [B, 257, 128]
    target = f["target"]  # [B, 256]
    per_core = []
    for c in range(NCORES):
        sl = slice(c * BL, (c + 1) * BL)
        # featT [f=128, t=257, b=128]
        featT = _bf(feat[sl].transpose(2, 1, 0))
        per_core.append({"featT": featT, "targetT": _f32(target[sl].T)})
    return p, per_core


def _build(enc_steps=TP, dec_steps=T, zero_unused=False, dbg=False):
    import concourse.mybir as mybir
    from concourse import bacc
    from concourse.tile import TileContext

    dt = mybir.dt
    AF = mybir.ActivationFunctionType
    OP = mybir.AluOpType

    nc = bacc.Bacc("TRN2")

    # ---- DRAM parameters ----
    dram = {}

    def din(name, shape, dtype):
        dram[name] = nc.declare_dram_parameter(name, list(shape), dtype, isOutput=False)

    din("featT", (128, TP, BL), dt.bfloat16)
    din("targetT", (T, BL), dt.float32)
    din("w_enc_a1", (128, 5, TP), dt.bfloat16)
    din("b_enc_a1", (128, 3), dt.float32)
    din("w_enc_a2", (128, 3, 128), dt.bfloat16)
    din("b_enc_a2", (128, 1), dt.float32)
    din("w_enc_g", (128, 3, 4 * HE), dt.bfloat16)
    din("b_enc_g", (128, 8), dt.float32)
    din("w_q", (128, 2, 2), dt.bfloat16)
    din("w_dec_a1", (128, 6, HE), dt.bfloat16)
    din("b_dec_a1", (128, 2), dt.float32)
    din("w_dec_a2", (128, 2, TP), dt.bfloat16)
    din("b_dec_a2", (128, 3), dt.float32)
    din("w_dec_g", (128, 3, 4 * HD), dt.bfloat16)
    din("b_dec_g", (128, 8), dt.float32)
    din("w_fh", (128, 2, 1), dt.bfloat16)
    din("consts", (1, 4), dt.float32)
    out_d = nc.declare_dram_parameter("out", [BL], dt.float32, isOutput=True)
    dbg_d = {}
    if dbg:
        for name, shape, dty in [
            ("dbg_hs0", [128, 8, BL], dt.bfloat16),
            ("dbg_q", [128, 3, 2, BL], dt.bfloat16),
            ("dbg_req", [128, 3, 2, BL], dt.bfloat16),
            ("dbg_zd", [1, 2, BL], dt.float32),
            ("dbg_rz", [1, BL], dt.float32),
            ("dbg_dcz", [1, BL], dt.float32),
            ("dbg_xi1", [1, BL], dt.float32),
            ("dbg_y", [1, BL], dt.float32),
            ("dbg_hd", [128, 2, BL], dt.bfloat16),
            ("dbg_aT", [128, 2, BL], dt.bfloat16),
            ("dbg_rf", [128, 3, BL], dt.bfloat16),
            ("dbg_fin", [1, 2, BL], dt.float32),
            ("dbg_dfz", [1, BL], dt.float32),
        ]:
            dbg_d[name] = nc.declare_dram_parameter(name, shape, dty, isOutput=True)

    with TileContext(nc) as tc:
        with (
            tc.tile_pool(name="consts", bufs=1) as cp,
            tc.tile_pool(name="state", bufs=1) as sp,
            tc.tile_pool(name="feat", bufs=8) as fp,
            tc.tile_pool(name="work", bufs=2) as wp,
            tc.tile_pool(name="lstm", bufs=2) as lp,
        ):
            # ---- load weights into SBUF ----
            sb = {}
            for name, shape, dty in [
                ("w_enc_a1", (128, 5, TP), dt.bfloat16),
                ("b_enc_a1", (128, 3), dt.float32),
                ("w_enc_a2", (128, 3, 128), dt.bfloat16),
                ("b_enc_a2", (128, 1), dt.float32),
                ("w_enc_g", (128, 3, 4 * HE), dt.bfloat16),
                ("b_enc_g", (128, 8), dt.float32),
                ("w_q", (128, 2, 2), dt.bfloat16),
                ("w_dec_a1", (128, 6, HE), dt.bfloat16),
                ("b_dec_a1", (128, 2), dt.float32),
                ("w_dec_a2", (128, 2, TP), dt.bfloat16),
                ("b_dec_a2", (128, 3), dt.float32),
                ("w_dec_g", (128, 3, 4 * HD), dt.bfloat16),
                ("b_dec_g", (128, 8), dt.float32),
                ("w_fh", (128, 2, 1), dt.bfloat16),
                ("consts", (1, 4), dt.float32),
            ]:
                t = cp.tile(list(shape), dty, tag=name)
                nc.sync.dma_start(out=t, in_=dram[name].ap())
                sb[name] = t

            ones_bf = cp.tile([128, 1], dt.bfloat16, tag="ones")
            nc.vector.memset(ones_bf, 1.0)
            zero_bf = cp.tile([128, BL], dt.bfloat16, tag="zero")
            nc.vector.memset(zero_bf, 0.0)

            # persistent big buffers
            hs0 = cp.tile([128, TP, BL], dt.bfloat16, tag="hs0")  # h feats 0:128
            hs1 = cp.tile([128, TP, BL], dt.bfloat16, tag="hs1")  # h feats 128:256
            qT = cp.tile([128, 3, 2, BL], dt.bfloat16, tag="qT")  # [t'%128, t'//128, {c,f}, b]
            if zero_unused:
                nc.vector.memset(hs0, 0.0)
                nc.vector.memset(hs1, 0.0)
                nc.vector.memset(qT, 0.0)

            # encoder state
            c_f = sp.tile([128, 2, BL], dt.float32, tag="c_f")
            c_b = sp.tile([128, 2, BL], dt.bfloat16, tag="c_b")
            nc.vector.memset(c_f, 0.0)
            nc.vector.memset(c_b, 0.0)

            with (
                tc.tile_pool(name="ps_a1", bufs=2, space="PSUM") as ps_a1,
                tc.tile_pool(name="ps_g", bufs=2, space="PSUM") as ps_g,
                tc.tile_pool(name="ps_q", bufs=2, space="PSUM") as ps_q,
            ):
                for t in range(enc_steps):
                    ft = fp.tile([128, BL], dt.bfloat16, tag="ft")
                    nc.sync.dma_start(out=ft, in_=dram["featT"].ap()[:, t, :])

                    if t == 0:
                        hp0, hp1 = zero_bf, zero_bf
                    else:
                        hp0, hp1 = hs0[:, t - 1, :], hs1[:, t - 1, :]
                    rhs_a1 = [ft, hp0, hp1, c_b[:, 0, :], c_b[:, 1, :]]

                    # attn1: aT [257 -> (128,128,1), b]
                    a_ps = ps_a1.tile([128, 4, BL], dt.float32, tag="a_ps")
                    for m, mm in enumerate((128, 128, 1)):
                        for k in range(5):
                            nc.tensor.matmul(
                                a_ps[:mm, m, :],
                                sb["w_enc_a1"][:, k, m * 128 : m * 128 + mm],
                                rhs_a1[k],
                                start=(k == 0),
                                stop=(k == 4),
                            )
                    aT = wp.tile([128, 3, BL], dt.bfloat16, tag="aT")
                    for m, mm in enumerate((128, 128, 1)):
                        nc.scalar.activation(
                            out=aT[:mm, m, :],
                            in_=a_ps[:mm, m, :],
                            func=AF.Tanh,
                            bias=sb["b_enc_a1"][:mm, m : m + 1],
                        )

                    # attn2 + xi = (alpha + ba2) * x_t
                    al_ps = a_ps[:, 3, :]
                    for k, kk in enumerate((128, 128, 1)):
                        nc.tensor.matmul(
                            al_ps,
                            sb["w_enc_a2"][:kk, k, :],
                            aT[:kk, k, :],
                            start=(k == 0),
                            stop=(k == 2),
                        )
                    xiT = wp.tile([128, BL], dt.bfloat16, tag="xiT")
                    nc.vector.scalar_tensor_tensor(
                        out=xiT,
                        in0=al_ps,
                        scalar=sb["b_enc_a2"][:, 0:1],
                        in1=ft,
                        op0=OP.add,
                        op1=OP.mult,
                    )

                    # gates
                    g_ps = ps_g.tile([128, 8, BL], dt.float32, tag="g_ps")
                    rhs_g = [xiT, hp0, hp1]
                    for j in (0, 2, 4, 6, 1, 3, 5, 7):
                        for k in range(3):
                            nc.tensor.matmul(
                                g_ps[:, j, :],
                                sb["w_enc_g"][:, k, j * 128 : (j + 1) * 128],
                                rhs_g[k],
                                start=(k == 0),
                                stop=(k == 2),
                            )

                    # LSTM elementwise per half, ops ordered to chase the
                    # gate matmul chunks (j emitted 0,2,4,6 then 1,3,5,7)
                    hdst = (hs0, hs1)
                    bg = sb["b_enc_g"]
                    for j2 in range(2):
                        si = lp.tile([128, BL], dt.float32, tag=f"si{j2}")
                        sf = lp.tile([128, BL], dt.float32, tag=f"sf{j2}")
                        tg = lp.tile([128, BL], dt.float32, tag=f"tg{j2}")
                        so = lp.tile([128, BL], dt.float32, tag=f"so{j2}")
                        nc.scalar.activation(out=si, in_=g_ps[:, 0 + j2, :], func=AF.Sigmoid, bias=bg[:, 0 + j2 : 1 + j2])
                        nc.scalar.activation(out=sf, in_=g_ps[:, 2 + j2, :], func=AF.Sigmoid, bias=bg[:, 2 + j2 : 3 + j2])
                        nc.scalar.activation(out=tg, in_=g_ps[:, 4 + j2, :], func=AF.Tanh, bias=bg[:, 4 + j2 : 5 + j2])
                        p1 = lp.tile([128, BL], dt.float32, tag=f"p1{j2}")
                        t2 = lp.tile([128, BL], dt.float32, tag=f"t2{j2}")
                        tc_ = lp.tile([128, BL], dt.float32, tag=f"tc{j2}")
                        nc.vector.tensor_mul(p1, si, tg)
                        nc.vector.tensor_mul(t2, sf, c_f[:, j2, :])
                        nc.vector.tensor_add(c_f[:, j2, :], t2, p1)
                        nc.vector.tensor_copy(c_b[:, j2, :], c_f[:, j2, :])
                        nc.scalar.activation(out=so, in_=g_ps[:, 6 + j2, :], func=AF.Sigmoid, bias=bg[:, 6 + j2 : 7 + j2])
                        nc.scalar.activation(out=tc_, in_=c_f[:, j2, :], func=AF.Tanh)
                        nc.vector.tensor_mul(hdst[j2][:, t, :], so, tc_)

                    # q rows: q_{c,f}[t] = h_t . W -> [2, b] -> DMA to qT row
                    q_ps = ps_q.tile([2, BL], dt.float32, tag="q_ps")
                    for k in range(2):
                        nc.tensor.matmul(
                            q_ps,
                            sb["w_q"][:, k, :],
                            hdst[k][:, t, :],
                            start=(k == 0),
                            stop=(k == 1),
                        )
                    q_row = fp.tile([2, BL], dt.bfloat16, tag="q_row")
                    nc.vector.tensor_copy(q_row, q_ps)
                    nc.sync.dma_start(
                        out=qT[t % 128 : t % 128 + 1, t // 128, :, :], in_=q_row
                    )

            # ================= decoder =================
            hdT = sp.tile([128, 2, BL], dt.bfloat16, tag="hdT")
            cd_f = sp.tile([128, 2, BL], dt.float32, tag="cd_f")
            cd_b = sp.tile([128, 2, BL], dt.bfloat16, tag="cd_b")
            nc.vector.memset(hdT, 0.0)
            nc.vector.memset(cd_f, 0.0)
            nc.vector.memset(cd_b, 0.0)
            o_sb = sp.tile([1, BL], dt.float32, tag="o_sb")

            with (
                tc.tile_pool(name="ps_da1", bufs=2, space="PSUM") as ps_da1,
                tc.tile_pool(name="ps_s", bufs=1, space="PSUM") as ps_s,
                tc.tile_pool(name="ps_zd", bufs=1, space="PSUM") as ps_zd,
                tc.tile_pool(name="ps_dg", bufs=2, space="PSUM") as ps_dg,
            ):
                for t in range(dec_steps):
                    rhs_a1 = [
                        hdT[:, 0, :],
                        hdT[:, 1, :],
                        cd_b[:, 0, :],
                        cd_b[:, 1, :],
                        hs0[:, t, :],
                        hs1[:, t, :],
                    ]
                    a_ps = ps_da1.tile([128, 2, BL], dt.float32, tag="da_ps")
                    for m in range(2):
                        for k in range(6):
                            nc.tensor.matmul(
                                a_ps[:, m, :],
                                sb["w_dec_a1"][:, k, m * 128 : (m + 1) * 128],
                                rhs_a1[k],
                                start=(k == 0),
                                stop=(k == 5),
                            )
                    aT = wp.tile([128, 2, BL], dt.bfloat16, tag="daT")
                    for m in range(2):
                        nc.scalar.activation(
                            out=aT[:, m, :],
                            in_=a_ps[:, m, :],
                            func=AF.Tanh,
                            bias=sb["b_dec_a1"][:, m : m + 1],
                        )

                    # attn2 scores s [257 -> (128,128,1), b]
                    s_ps = ps_s.tile([128, 3, BL], dt.float32, tag="s_ps")
                    for m, mm in enumerate((128, 128, 1)):
                        for k in range(2):
                            nc.tensor.matmul(
                                s_ps[:mm, m, :],
                                sb["w_dec_a2"][:, k, m * 128 : m * 128 + mm],
                                aT[:, k, :],
                                start=(k == 0),
                                stop=(k == 1),
                            )

                    # e = exp(s + ba2); eq = e * q_c
                    req = wp.tile([128, 3, 2, BL], dt.bfloat16, tag="req")
                    for m, mm in enumerate((128, 128, 1)):
                        nc.scalar.activation(
                            out=req[:mm, m, 0, :],
                            in_=s_ps[:mm, m, :],
                            func=AF.Exp,
                            bias=sb["b_dec_a2"][:mm, m : m + 1],
                        )
                    for m, mm in enumerate((128, 128, 1)):
                        nc.vector.tensor_mul(
                            req[:mm, m, 1, :], req[:mm, m, 0, :], qT[:mm, m, 0, :]
                        )

                    # [Z | dot_c] = ones . [e | eq]
                    zd_ps = ps_zd.tile([1, 2, BL], dt.float32, tag="zd_ps")
                    for m, mm in enumerate((128, 128, 1)):
                        nc.tensor.matmul(
                            zd_ps,
                            ones_bf[:mm, :],
                            req[:mm, m, :, :],
                            start=(m == 0),
                            stop=(m == 2),
                        )

                    # xi = y_t*Wc00 + dot_c/Z + bc
                    zd_cp = wp.tile([1, 2, BL], dt.float32, tag="zd_cp")
                    if dbg:
                        nc.vector.tensor_copy(zd_cp, zd_ps)
                    rz = wp.tile([1, BL], dt.float32, tag="rz")
                    dcz = wp.tile([1, BL], dt.float32, tag="dcz")
                    xi1 = wp.tile([1, BL], dt.float32, tag="xi1")
                    xiT = wp.tile([1, BL], dt.bfloat16, tag="dxiT")
                    nc.vector.reciprocal(rz, zd_ps[0:1, 0, :])
                    nc.vector.tensor_mul(dcz, zd_ps[0:1, 1, :], rz)
                    y_row = fp.tile([1, BL], dt.float32, tag="y_row")
                    nc.sync.dma_start(out=y_row, in_=dram["targetT"].ap()[t : t + 1, :])
                    nc.vector.scalar_tensor_tensor(
                        out=xi1,
                        in0=y_row,
                        scalar=sb["consts"][0:1, 2:3],
                        in1=dcz,
                        op0=OP.mult,
                        op1=OP.add,
                    )
                    nc.vector.tensor_scalar(
                        out=xiT,
                        in0=xi1,
                        scalar1=sb["consts"][0:1, 0:1],
                        scalar2=None,
                        op0=OP.add,
                    )

                    # gates
                    g_ps = ps_dg.tile([128, 8, BL], dt.float32, tag="dg_ps")
                    for j in (0, 2, 4, 6, 1, 3, 5, 7):
                        nc.tensor.matmul(
                            g_ps[:, j, :],
                            sb["w_dec_g"][0:1, 0, j * 128 : (j + 1) * 128],
                            xiT,
                            start=True,
                            stop=False,
                        )
                        for k in (1, 2):
                            nc.tensor.matmul(
                                g_ps[:, j, :],
                                sb["w_dec_g"][:, k, j * 128 : (j + 1) * 128],
                                hdT[:, k - 1, :],
                                start=False,
                                stop=(k == 2),
                            )

                    bg = sb["b_dec_g"]
                    for j2 in range(2):
                        si = lp.tile([128, BL], dt.float32, tag=f"dsi{j2}")
                        sf = lp.tile([128, BL], dt.float32, tag=f"dsf{j2}")
                        tg = lp.tile([128, BL], dt.float32, tag=f"dtg{j2}")
                        so = lp.tile([128, BL], dt.float32, tag=f"dso{j2}")
                        nc.scalar.activation(out=si, in_=g_ps[:, 0 + j2, :], func=AF.Sigmoid, bias=bg[:, 0 + j2 : 1 + j2])
                        nc.scalar.activation(out=sf, in_=g_ps[:, 2 + j2, :], func=AF.Sigmoid, bias=bg[:, 2 + j2 : 3 + j2])
                        nc.scalar.activation(out=tg, in_=g_ps[:, 4 + j2, :], func=AF.Tanh, bias=bg[:, 4 + j2 : 5 + j2])
                        p1 = lp.tile([128, BL], dt.float32, tag=f"dp1{j2}")
                        t2 = lp.tile([128, BL], dt.float32, tag=f"dt2{j2}")
                        tc_ = lp.tile([128, BL], dt.float32, tag=f"dtc{j2}")
                        nc.vector.tensor_mul(p1, si, tg)
                        nc.vector.tensor_mul(t2, sf, cd_f[:, j2, :])
                        nc.vector.tensor_add(cd_f[:, j2, :], t2, p1)
                        nc.vector.tensor_copy(cd_b[:, j2, :], cd_f[:, j2, :])
                        nc.scalar.activation(out=so, in_=g_ps[:, 6 + j2, :], func=AF.Sigmoid, bias=bg[:, 6 + j2 : 7 + j2])
                        nc.scalar.activation(out=tc_, in_=cd_f[:, j2, :], func=AF.Tanh)
                        nc.vector.tensor_mul(hdT[:, j2, :], so, tc_)

                    if t == dec_steps - 1:
                        # dot_f = ones . (e * q_f);  hw = hd . Wf[:, :HD]
                        rf = wp.tile([128, 3, BL], dt.bfloat16, tag="rf")
                        for m, mm in enumerate((128, 128, 1)):
                            nc.vector.tensor_mul(
                                rf[:mm, m, :], req[:mm, m, 0, :], qT[:mm, m, 1, :]
                            )
                        fin_ps = ps_zd.tile([1, 2, BL], dt.float32, tag="zd_ps")
                        for m, mm in enumerate((128, 128, 1)):
                            nc.tensor.matmul(
                                fin_ps[0:1, 0, :],
                                ones_bf[:mm, :],
                                rf[:mm, m, :],
                                start=(m == 0),
                                stop=(m == 2),
                            )
                        for k in range(2):
                            nc.tensor.matmul(
                                fin_ps[0:1, 1, :],
                                sb["w_fh"][:, k, :],
                                hdT[:, k, :],
                                start=(k == 0),
                                stop=(k == 1),
                            )
                        dfz = wp.tile([1, BL], dt.float32, tag="dfz")
                        nc.vector.tensor_mul(dfz, fin_ps[0:1, 0, :], rz)
                        if dbg:
                            fin_cp = wp.tile([1, 2, BL], dt.float32, tag="fin_cp")
                            nc.vector.tensor_copy(fin_cp, fin_ps)
                            nc.sync.dma_start(out=dbg_d["dbg_rf"].ap(), in_=rf)
                            nc.sync.dma_start(out=dbg_d["dbg_fin"].ap(), in_=fin_cp)
                            nc.sync.dma_start(out=dbg_d["dbg_dfz"].ap(), in_=dfz)
                        hw_sb = wp.tile([1, BL], dt.float32, tag="hw_sb")
                        nc.vector.tensor_copy(hw_sb, fin_ps[0:1, 1, :])
                        nc.vector.scalar_tensor_tensor(
                            out=o_sb,
                            in0=hw_sb,
                            scalar=sb["consts"][0:1, 1:2],
                            in1=dfz,
                            op0=OP.add,
                            op1=OP.add,
                        )
                        nc.sync.dma_start(out=out_d.ap(), in_=o_sb[0:1, :])
                        if dbg:
                            nc.sync.dma_start(out=dbg_d["dbg_hs0"].ap(), in_=hs0[:, 0:8, :])
                            nc.sync.dma_start(out=dbg_d["dbg_q"].ap(), in_=qT)
                            nc.sync.dma_start(out=dbg_d["dbg_req"].ap(), in_=req)
                            nc.sync.dma_start(out=dbg_d["dbg_zd"].ap(), in_=zd_cp)
                            nc.sync.dma_start(out=dbg_d["dbg_rz"].ap(), in_=rz)
                            nc.sync.dma_start(out=dbg_d["dbg_dcz"].ap(), in_=dcz)
                            nc.sync.dma_start(out=dbg_d["dbg_xi1"].ap(), in_=xi1)
                            nc.sync.dma_start(out=dbg_d["dbg_y"].ap(), in_=y_row)
                            nc.sync.dma_start(out=dbg_d["dbg_hd"].ap(), in_=hdT)
                            nc.sync.dma_start(out=dbg_d["dbg_aT"].ap(), in_=aT)

    nc.finalize()
    return nc


def _get_nc():
    if "nc" not in _CACHE:
        _CACHE["nc"] = _build()
    return _CACHE["nc"]


def _run(inputs, **kw):
    from concourse.bass_utils import run_bass_kernel_spmd

    shared, per_core = _pack_inputs(inputs)
    nc = _get_nc()
    in_maps = []
    for c in range(NCORES):
        m = dict(shared)
        m.update(per_core[c])
        in_maps.append(m)
    res = run_bass_kernel_spmd(nc, in_maps, list(range(NCORES)), **kw)
    out = np.concatenate([np.asarray(res.results[c]["out"]) for c in range(NCORES)])
    return out.astype(np.float32).reshape(B, 1), res


def kernel(**inputs):
    return _run(inputs)[0]



# revision 40
# speedup vs baseline: 1.4632x; 1.4632x over previous
"""DARNN (dual-attention RNN) Trainium2 Bass kernel — v2.

Strategy (pure data parallel, 8 cores, B=1024 -> 128 samples/core):

Activations are feature-major on-chip: x[b, f] lives in SBUF as xT[f, b]
(features on partitions, local batch on the free dim). Matmuls contract
over partitions with pre-transposed bf16 weights stationary.

v2 changes vs v1 baseline:
- Decoder softmax is batch-major: attn2 uses the (feature-major) tanh
  output as the *stationary* operand, producing scores [b, 257] in one
  PSUM bank (bias folded in via a ones-row matmul). A single Exp
  activation with accum_out yields e and Z; tensor_tensor_reduce yields
  dot_c; reciprocal runs at FD=1. xi returns to row layout via one PE
  transpose.
- All decoder sigmoids are rewritten as 0.5*tanh(x/2)+0.5 with the 0.5/2
  factors folded into packed weights (h'=2h, c'=2c representation), so
  the decoder only ever uses {Tanh, Exp} -> zero ACT table reloads.
- Decoder gate biases (+ comb_fc bias) are folded into a k=2 [xi; 1]
  matmul chunk, enabling two wide FD=512 gate activations.
- Fine-grained PSUM tiles (per attn1 m-chunk, per gate) let ACT start as
  soon as its slice is accumulated instead of after the whole block.
- PE issue order hides tanh/softmax latency behind h-only gate matmuls;
  the decoder attn1 feat contribution for step t+1 is prefilled into
  PSUM during step t's LSTM tail.
- Encoder hidden states live in one hs[128, 257, 2, BL] tile so the
  LSTM elementwise runs wide (FD=256) across both feature halves.
"""

import numpy as np
import ml_dtypes

B, T, NF, HE, HD = 1024, 256, 128, 256, 256
NCORES = 8
BL = B // NCORES  # 128 local batch
TP = T + 1  # 257
BF16 = ml_dtypes.bfloat16

_CACHE = {}


def _bf(x):
    return np.ascontiguousarray(np.asarray(x).astype(BF16))


def _f32(x):
    return np.ascontiguousarray(np.asarray(x).astype(np.float32))


def _pack_inputs(inputs):
    """Pack weights/biases into per-SBUF-tile layouts (shared across cores)."""
    f = {k: np.asarray(v, dtype=np.float32) for k, v in inputs.items()}
    p = {}

    # --- encoder attn1: a = tanh(cat[x,h,c] @ Wa1.T + ba1) ---
    # lhsT tile [128, 5, 257]: [p, k, m] = Wa1[m, k*128+p]
    wa1 = f["enc_Wa1"]  # [257, 640]
    p["w_enc_a1"] = _bf(wa1.T.reshape(5, 128, TP).transpose(1, 0, 2))
    b1 = np.zeros((128, 3), np.float32)
    b1.T.flat[:TP] = f["enc_ba1"]  # [p, j] = ba1[j*128+p]
    p["b_enc_a1"] = _f32(b1)

    # --- encoder attn2: alpha = a @ Wa2.T + ba2 ---  K=257 (3 chunks), M=128
    wa2 = f["enc_Wa2"]  # [128, 257]
    w = np.zeros((128, 3, 128), np.float32)
    w2 = wa2.T  # [257, 128]
    w[:, 0, :] = w2[0:128]
    w[:, 1, :] = w2[128:256]
    w[0, 2, :] = w2[256]
    p["w_enc_a2"] = _bf(w)
    p["b_enc_a2"] = _f32(f["enc_ba2"].reshape(128, 1))

    # --- encoder gates: g = Wih@xi + Whh@h + b ---  K chunks: [xi, h0, h1]
    # column order j = [i0,i1,f0,f1,g0,g1,o0,o1] (natural torch order)
    w = np.zeros((128, 3, 4 * HE), np.float32)
    w[:, 0, :] = f["enc_Wih"].T  # [128, 1024]
    w[:, 1, :] = f["enc_Whh"].T[0:128]
    w[:, 2, :] = f["enc_Whh"].T[128:256]
    p["w_enc_g"] = _bf(w)
    bsum = f["enc_bih"] + f["enc_bhh"]
    p["b_enc_g"] = _f32(bsum.reshape(8, 128).T)  # [p, j] = b[j*128+p]

    # --- q projections: q_c = h . Wc[0,1:], q_f = h . Wf[0,HD:] ---
    w = np.zeros((128, 2, 2), np.float32)
    w[:, 0, 0] = f["dec_Wc"][0, 1 : 1 + 128]
    w[:, 1, 0] = f["dec_Wc"][0, 129 : 1 + 256]
    w[:, 0, 1] = f["dec_Wf"][0, HD : HD + 128]
    w[:, 1, 1] = f["dec_Wf"][0, HD + 128 : HD + 256]
    p["w_q"] = _bf(w)

    # --- decoder attn1: a = tanh(cat[h,c,feat] @ Wa1.T + ba1) --- K=768 (6)
    # decoder h,c are stored as h'=2h, c'=2c -> scale those k-chunks by 0.5
    wa1d = f["dec_Wa1"].copy()  # [256, 768]
    wa1d[:, 0:512] *= 0.5  # h and c columns
    p["w_dec_a1"] = _bf(wa1d.T.reshape(6, 128, HE).transpose(1, 0, 2))
    p["b_dec_a1"] = _f32(f["dec_ba1"].reshape(2, 128).T)

    # --- decoder attn2 (batch-major): s[b, t'] = a.T @ Wa2dT + ba2 ---
    # moving operand [p=feat chunk, k, n=t'] = Wa2d[n, k*128+p]
    wa2d = f["dec_Wa2"]  # [257, 256]
    p["w_dec_a2"] = _bf(wa2d.T.reshape(2, 128, TP).transpose(1, 0, 2))
    ba2r = np.zeros((1, TP), np.float32)
    ba2r[0, :] = f["dec_ba2"]
    p["ba2_row"] = _bf(ba2r)

    # --- decoder gates ---
    # tanh-form LSTM: i,f,o gates become tanh(0.5*(pre+b)); g stays tanh.
    # Fold: h' = 2h -> Whh columns *0.5 ; g-gate pre-act scaled *2 so a
    # uniform ACT scale=0.5 works for the whole tile.
    sgate = np.ones((4 * HD,), np.float32)
    sgate[512:768] = 2.0  # g-gate columns
    whh = f["dec_Whh"].T * 0.5  # [256, 1024] (h' fold)
    w = np.zeros((128, 2, 4 * HD), np.float32)
    w[:, 0, :] = whh[0:128] * sgate
    w[:, 1, :] = whh[128:256] * sgate
    p["w_dec_g"] = _bf(w)
    # k=2 chunk [xi_nc; 1] with xi_nc = y*Wc00 + dot_c/Z  (bc folded here)
    wih = f["dec_Wih"][:, 0]  # [1024]
    bsum = f["dec_bih"] + f["dec_bhh"] + wih * f["dec_bc"][0]
    gx = np.zeros((2, 4 * HD), np.float32)
    gx[0, :] = wih * sgate
    gx[1, :] = bsum * sgate
    p["w_dec_gx"] = _bf(gx)

    # --- final: out = hd . Wf[0,:HD] + dot_f/Z + bf ---  (hd' = 2hd fold)
    w = np.zeros((128, 2, 1), np.float32)
    w[:, 0, 0] = f["dec_Wf"][0, 0:128] * 0.5
    w[:, 1, 0] = f["dec_Wf"][0, 128:256] * 0.5
    p["w_fh"] = _bf(w)

    # --- broadcast consts: [bc, bf, Wc00, 0] replicated over partitions ---
    cb = np.zeros((128, 4), np.float32)
    cb[:, 0] = f["dec_bc"][0]
    cb[:, 1] = f["dec_bf"][0]
    cb[:, 2] = f["dec_Wc"][0, 0]
    p["consts_b"] = _f32(cb)

    # --- identity for PE transposes ---
    p["ident"] = _bf(np.eye(128, dtype=np.float32))

    # --- per-core tensors ---
    feat = f["feat"]  # [B, 257, 128]
    target = f["target"]  # [B, 256]
    per_core = []
    for c in range(NCORES):
        sl = slice(c * BL, (c + 1) * BL)
        featT = _bf(feat[sl].transpose(2, 1, 0))  # [f=128, t=257, b=128]
        per_core.append({"featT": featT, "targetB": _f32(target[sl])})
    return p, per_core


def _build(enc_steps=TP, dec_steps=T):
    import concourse.mybir as mybir
    from concourse import bacc
    from concourse.tile import TileContext

    dt = mybir.dt
    AF = mybir.ActivationFunctionType
    OP = mybir.AluOpType

    nc = bacc.Bacc("TRN2")

    # ---- DRAM parameters ----
    dram = {}

    def din(name, shape, dtype):
        dram[name] = nc.declare_dram_parameter(name, list(shape), dtype, isOutput=False)

    din("featT", (128, TP, BL), dt.bfloat16)
    din("targetB", (BL, T), dt.float32)
    din("w_enc_a1", (128, 5, TP), dt.bfloat16)
    din("b_enc_a1", (128, 3), dt.float32)
    din("w_enc_a2", (128, 3, 128), dt.bfloat16)
    din("b_enc_a2", (128, 1), dt.float32)
    din("w_enc_g", (128, 3, 4 * HE), dt.bfloat16)
    din("b_enc_g", (128, 8), dt.float32)
    din("w_q", (128, 2, 2), dt.bfloat16)
    din("w_dec_a1", (128, 6, HE), dt.bfloat16)
    din("b_dec_a1", (128, 2), dt.float32)
    din("w_dec_a2", (128, 2, TP), dt.bfloat16)
    din("ba2_row", (1, TP), dt.bfloat16)
    din("w_dec_g", (128, 2, 4 * HD), dt.bfloat16)
    din("w_dec_gx", (2, 4 * HD), dt.bfloat16)
    din("w_fh", (128, 2, 1), dt.bfloat16)
    din("consts_b", (128, 4), dt.float32)
    din("ident", (128, 128), dt.bfloat16)
    out_d = nc.declare_dram_parameter("out", [BL], dt.float32, isOutput=True)

    with TileContext(nc) as tc:
        with (
            tc.tile_pool(name="consts", bufs=1) as cp,
            tc.tile_pool(name="state", bufs=1) as sp,
            tc.tile_pool(name="feat", bufs=8) as fp,
            tc.tile_pool(name="work", bufs=2) as wp,
        ):
            # ---- load weights into SBUF ----
            sb = {}
            for name, shape, dty in [
                ("w_enc_a1", (128, 5, TP), dt.bfloat16),
                ("b_enc_a1", (128, 3), dt.float32),
                ("w_enc_a2", (128, 3, 128), dt.bfloat16),
                ("b_enc_a2", (128, 1), dt.float32),
                ("w_enc_g", (128, 3, 4 * HE), dt.bfloat16),
                ("b_enc_g", (128, 8), dt.float32),
                ("w_q", (128, 2, 2), dt.bfloat16),
                ("w_dec_a1", (128, 6, HE), dt.bfloat16),
                ("b_dec_a1", (128, 2), dt.float32),
                ("w_dec_a2", (128, 2, TP), dt.bfloat16),
                ("ba2_row", (1, TP), dt.bfloat16),
                ("w_dec_g", (128, 2, 4 * HD), dt.bfloat16),
                ("w_dec_gx", (2, 4 * HD), dt.bfloat16),
                ("w_fh", (128, 2, 1), dt.bfloat16),
                ("consts_b", (128, 4), dt.float32),
                ("ident", (128, 128), dt.bfloat16),
                ("targetB", (BL, T), dt.float32),
            ]:
                t = cp.tile(list(shape), dty, tag=name)
                nc.sync.dma_start(out=t, in_=dram[name].ap())
                sb[name] = t

            ones_row = cp.tile([1, 128], dt.bfloat16, tag="ones_row")
            nc.vector.memset(ones_row, 1.0)
            zero_bf = cp.tile([128, BL], dt.bfloat16, tag="zero")
            nc.vector.memset(zero_bf, 0.0)

            # persistent big buffers
            hs = cp.tile([128, TP, 2, BL], dt.bfloat16, tag="hs")  # [f, t, half, b]
            qT = cp.tile([128, 3, 2, BL], dt.bfloat16, tag="qT")  # [t'%128, t'//128, {c,f}, b]
            nc.vector.memset(qT, 0.0)
            qcT = cp.tile([128, 384], dt.bfloat16, tag="qcT")  # [b, t'] (padded)
            qfT = cp.tile([128, 384], dt.bfloat16, tag="qfT")

            # encoder state
            c_f = sp.tile([128, 2, BL], dt.float32, tag="c_f")
            c_b = sp.tile([128, 2, BL], dt.bfloat16, tag="c_b")
            nc.vector.memset(c_f, 0.0)
            nc.vector.memset(c_b, 0.0)

            # ================= encoder =================
            with (
                tc.tile_pool(name="ps_a1", bufs=2, space="PSUM") as ps_a1,
                tc.tile_pool(name="ps_g", bufs=2, space="PSUM") as ps_g,
                tc.tile_pool(name="ps_q", bufs=1, space="PSUM") as ps_q,
            ):
                for t in range(enc_steps):
                    ft = fp.tile([128, BL], dt.bfloat16, tag="ft")
                    nc.sync.dma_start(out=ft, in_=dram["featT"].ap()[:, t, :])

                    if t == 0:
                        hp0, hp1 = zero_bf, zero_bf
                    else:
                        hp0, hp1 = hs[:, t - 1, 0, :], hs[:, t - 1, 1, :]
                    rhs_a1 = [ft, hp0, hp1, c_b[:, 0, :], c_b[:, 1, :]]
                    rhs_g = [None, hp0, hp1]

                    # attn1: aT [257 -> (128,128,1), b] ; one bank [m0,m1,m2,al]
                    a1 = ps_a1.tile([128, 4, BL], dt.float32, tag="a1", name="a1")
                    a1m = [a1[:, 0, :], a1[:, 1, :], a1[:1, 2, :]]
                    for m, mm in enumerate((128, 128, 1)):
                        for k in range(5):
                            nc.tensor.matmul(
                                a1m[m],
                                sb["w_enc_a1"][:, k, m * 128 : m * 128 + mm],
                                rhs_a1[k],
                                start=(k == 0),
                                stop=(k == 4),
                            )

                    # gates h-part early: fills PE while ACT does tanh
                    g_if = ps_g.tile([128, 4, BL], dt.float32, tag="g_if", name="g_if")
                    g_go = ps_g.tile([128, 4, BL], dt.float32, tag="g_go", name="g_go")
                    gsl = [g_if[:, 0:2, :], g_if[:, 2:4, :], g_go[:, 0:2, :], g_go[:, 2:4, :]]
                    # h-part: ONE group per bank spanning its 4 slots, so the
                    # late xi accumulates (below) see has_written bits intact
                    for g in range(4):
                        for half in range(2):
                            for k in (1, 2):
                                j = g * 2 + half
                                first = (g % 2, half, k) == (0, 0, 1)
                                last = (g % 2, half, k) == (1, 1, 2)
                                nc.tensor.matmul(
                                    gsl[g][:, half, :],
                                    sb["w_enc_g"][:, k, j * 128 : (j + 1) * 128],
                                    rhs_g[k],
                                    start=first,
                                    stop=last,
                                )

                    # tanh (ACT) per m-chunk
                    aT = [
                        wp.tile([128, BL], dt.bfloat16, tag="aT0", name="aT0"),
                        wp.tile([128, BL], dt.bfloat16, tag="aT1", name="aT1"),
                        wp.tile([1, BL], dt.bfloat16, tag="aT2", name="aT2"),
                    ]
                    for m, mm in enumerate((128, 128, 1)):
                        nc.scalar.activation(
                            out=aT[m][:mm, :],
                            in_=a1m[m][:mm, :],
                            func=AF.Tanh,
                            bias=sb["b_enc_a1"][:mm, m : m + 1],
                        )

                    # attn2 + xi = (alpha + ba2) * x_t
                    al_ps = a1[:, 3, :]
                    for k, kk in enumerate((128, 128, 1)):
                        nc.tensor.matmul(
                            al_ps,
                            sb["w_enc_a2"][:kk, k, :],
                            aT[k][:kk, :],
                            start=(k == 0),
                            stop=(k == 2),
                        )
                    xiT = wp.tile([128, BL], dt.bfloat16, tag="xiT")
                    nc.vector.scalar_tensor_tensor(
                        out=xiT,
                        in0=al_ps,
                        scalar=sb["b_enc_a2"][:, 0:1],
                        in1=ft,
                        op0=OP.add,
                        op1=OP.mult,
                    )

                    # gates xi-part: late accumulate into the closed groups
                    # (has_written bits already set -> HW accumulates; the
                    # group check is sim-only bookkeeping)
                    for g in range(4):
                        for half in range(2):
                            j = g * 2 + half
                            nc.tensor.matmul(
                                gsl[g][:, half, :],
                                sb["w_enc_g"][:, 0, j * 128 : (j + 1) * 128],
                                xiT,
                                start=False,
                                stop=True,
                                skip_group_check=True,
                            )

                    # LSTM elementwise; per-(gate,half) ACT starts as soon as
                    # that gate's psum closes, wide (FD=256) VEC ops after.
                    bg = sb["b_enc_g"]
                    sg = [
                        wp.tile([128, 2, BL], dt.float32, tag=f"s{n}", name=f"s{n}")
                        for n in "ifgo"
                    ]
                    for g, fn in ((0, AF.Sigmoid), (1, AF.Sigmoid), (2, AF.Tanh)):
                        for half in range(2):
                            j = g * 2 + half
                            nc.scalar.activation(
                                out=sg[g][:, half, :],
                                in_=gsl[g][:, half, :],
                                func=fn,
                                bias=bg[:, j : j + 1],
                            )
                    p1 = wp.tile([128, 2, BL], dt.float32, tag="p1")
                    t2 = wp.tile([128, 2, BL], dt.float32, tag="t2")
                    nc.vector.tensor_mul(p1, sg[0], sg[2])
                    nc.vector.tensor_mul(t2, sg[1], c_f)
                    nc.vector.tensor_add(c_f, t2, p1)
                    nc.vector.tensor_copy(c_b, c_f)
                    for half in range(2):
                        j = 6 + half
                        nc.scalar.activation(
                            out=sg[3][:, half, :],
                            in_=gsl[3][:, half, :],
                            func=AF.Sigmoid,
                            bias=bg[:, j : j + 1],
                        )
                    tc_ = wp.tile([128, 2, BL], dt.float32, tag="tc")
                    nc.scalar.activation(out=tc_, in_=c_f, func=AF.Tanh)
                    nc.vector.tensor_mul(hs[:, t, :, :], sg[3], tc_)

                    # q rows: q_{c,f}[t] = h_t . W -> [2, b] -> DMA to qT row
                    q_ps = ps_q.tile([2, BL], dt.float32, tag="q_ps")
                    for k in range(2):
                        nc.tensor.matmul(
                            q_ps,
                            sb["w_q"][:, k, :],
                            hs[:, t, k, :],
                            start=(k == 0),
                            stop=(k == 1),
                        )
                    q_row = fp.tile([2, BL], dt.bfloat16, tag="q_row")
                    nc.vector.tensor_copy(q_row, q_ps)
                    nc.sync.dma_start(
                        out=qT[t % 128 : t % 128 + 1, t // 128, :, :], in_=q_row
                    )

            # ======= transpose q to batch-major qcT/qfT [b, t'] =======
            with tc.tile_pool(name="ps_tr", bufs=2, space="PSUM") as ps_tr:
                for chunk in range(3):
                    for cf in range(2):
                        tp_ps = ps_tr.tile([128, 128], dt.bfloat16, tag="tp")
                        nc.tensor.transpose(
                            tp_ps, qT[:, chunk, cf, :], sb["ident"]
                        )
                        dst = qcT if cf == 0 else qfT
                        nc.vector.tensor_copy(
                            dst[:, chunk * 128 : (chunk + 1) * 128], tp_ps
                        )

            # ================= decoder =================
            hdT = sp.tile([128, 2, BL], dt.bfloat16, tag="hdT")  # h' = 2h
            cd_f = sp.tile([128, 2, BL], dt.float32, tag="cd_f")  # c' = 2c
            cd_b = sp.tile([128, 2, BL], dt.bfloat16, tag="cd_b")
            nc.vector.memset(hdT, 0.0)
            nc.vector.memset(cd_f, 0.0)
            nc.vector.memset(cd_b, 0.0)
            xiT2 = sp.tile([2, BL], dt.bfloat16, tag="xiT2")  # [xi_nc; 1]
            nc.vector.memset(xiT2, 1.0)  # row 1 stays ones
            o_col = sp.tile([128, 1], dt.float32, tag="o_col")

            with (
                tc.tile_pool(name="ps_da1", bufs=2, space="PSUM") as ps_da1,
                tc.tile_pool(name="ps_s", bufs=2, space="PSUM") as ps_s,
                tc.tile_pool(name="ps_dg", bufs=1, space="PSUM") as ps_dg,
                tc.tile_pool(name="ps_dx", bufs=1, space="PSUM") as ps_dx,
            ):
                nxt = {}

                def prefill(t):
                    da1 = ps_da1.tile([128, 2, BL], dt.float32, tag="da1", name="da1")
                    for m in range(2):
                        for k in range(2):  # feat chunks (4, 5)
                            nc.tensor.matmul(
                                da1[:, m, :],
                                sb["w_dec_a1"][:, 4 + k, m * 128 : (m + 1) * 128],
                                hs[:, t, k, :],
                                start=(m == 0 and k == 0),
                                stop=(m == 1 and k == 1),
                            )
                    nxt[t] = da1

                prefill(0)

                for t in range(dec_steps):
                    da1 = nxt.pop(t)
                    rhs_a1 = [
                        hdT[:, 0, :],
                        hdT[:, 1, :],
                        cd_b[:, 0, :],
                        cd_b[:, 1, :],
                    ]
                    for m in range(2):
                        for k in range(4):
                            nc.tensor.matmul(
                                da1[:, m, :],
                                sb["w_dec_a1"][:, k, m * 128 : (m + 1) * 128],
                                rhs_a1[k],
                                start=False,
                                stop=(k == 3),
                                skip_group_check=True,
                            )

                    # gates h-part early (only needs hdT)
                    g_if = ps_dg.tile([128, 4, BL], dt.float32, tag="g_if")
                    g_go = ps_dg.tile([128, 4, BL], dt.float32, tag="g_go")
                    for tile_, j0 in ((g_if, 0), (g_go, 4)):
                        for jl in range(4):
                            j = j0 + jl
                            for k in range(2):
                                nc.tensor.matmul(
                                    tile_[:, jl, :],
                                    sb["w_dec_g"][:, k, j * 128 : (j + 1) * 128],
                                    hdT[:, k, :],
                                    start=(jl == 0 and k == 0),
                                    stop=(jl == 3 and k == 1),
                                )

                    # attn1 tanh
                    aTd = wp.tile([128, 2, BL], dt.bfloat16, tag="aTd")
                    for m in range(2):
                        nc.scalar.activation(
                            out=aTd[:, m, :],
                            in_=da1[:, m, :],
                            func=AF.Tanh,
                            bias=sb["b_dec_a1"][:, m : m + 1],
                        )

                    # attn2, batch-major: s[b, t'] ; bias via ones-row matmul
                    s_ps = ps_s.tile(
                        [128, TP], dt.float32, tag="s_ps", padded_shape=[128, 512]
                    )
                    nc.tensor.matmul(
                        s_ps, ones_row, sb["ba2_row"], start=True, stop=False
                    )
                    for k in range(2):
                        nc.tensor.matmul(
                            s_ps,
                            aTd[:, k, :],
                            sb["w_dec_a2"][:, k, :],
                            start=False,
                            stop=(k == 1),
                        )

                    # softmax pieces: e, Z, dot_c, xi
                    e_sb = wp.tile([128, TP], dt.bfloat16, tag="e_sb")
                    z_t = wp.tile([128, 1], dt.float32, tag="z_t")
                    nc.scalar.activation(
                        out=e_sb, in_=s_ps, func=AF.Exp, accum_out=z_t
                    )
                    rz = wp.tile([128, 1], dt.float32, tag="rz")
                    nc.vector.reciprocal(rz, z_t)
                    eq = wp.tile([128, TP], dt.bfloat16, tag="eq")
                    dot_c = wp.tile([128, 1], dt.float32, tag="dot_c")
                    nc.vector.tensor_mul(eq, e_sb, qcT[:, 0:TP])
                    nc.vector.tensor_reduce(
                        out=dot_c, in_=eq, axis=mybir.AxisListType.X, op=OP.add
                    )
                    dcz = wp.tile([128, 1], dt.float32, tag="dcz")
                    nc.vector.tensor_mul(dcz, dot_c, rz)
                    xi_col = wp.tile([128, 1], dt.bfloat16, tag="xi_col")
                    nc.vector.scalar_tensor_tensor(
                        out=xi_col,
                        in0=sb["targetB"][:, t : t + 1],
                        scalar=sb["consts_b"][:, 2:3],
                        in1=dcz,
                        op0=OP.mult,
                        op1=OP.add,
                    )
                    # xi back to row layout: regular matmul against identity
                    # (out[0, n] = sum_k xi[k] * I[k, n] = xi[n])
                    xi_ps = ps_dx.tile([1, 128], dt.float32, tag="xi_ps")
                    nc.tensor.matmul(
                        xi_ps, xi_col, sb["ident"], start=True, stop=True
                    )
                    nc.vector.tensor_copy(xiT2[0:1, :], xi_ps[0:1, :])

                    # gates [xi; 1] chunk (bias folded): late accumulate
                    for tile_, j0 in ((g_if, 0), (g_go, 4)):
                        for jl in range(4):
                            j = j0 + jl
                            nc.tensor.matmul(
                                tile_[:, jl, :],
                                sb["w_dec_gx"][:, j * 128 : (j + 1) * 128],
                                xiT2,
                                start=False,
                                stop=True,
                                skip_group_check=True,
                            )

                    # prefill next step's attn1 feat part during the tail
                    if t + 1 < dec_steps:
                        prefill(t + 1)

                    # LSTM elementwise (tanh-form), wide ops
                    th_if = wp.tile([128, 4, BL], dt.float32, tag="th_if")
                    th_go = wp.tile([128, 4, BL], dt.float32, tag="th_go")
                    nc.scalar.activation(
                        out=th_if, in_=g_if, func=AF.Tanh, scale=0.5
                    )
                    nc.scalar.activation(
                        out=th_go, in_=g_go, func=AF.Tanh, scale=0.5
                    )
                    p1 = wp.tile([128, 2, BL], dt.float32, tag="dp1")
                    t2 = wp.tile([128, 2, BL], dt.float32, tag="dt2")
                    # p1 = (th_i + 1) * th_g ; t2 = (th_f + 1) * c'
                    nc.vector.scalar_tensor_tensor(
                        out=p1,
                        in0=th_if[:, 0:2, :],
                        scalar=1.0,
                        in1=th_go[:, 0:2, :],
                        op0=OP.add,
                        op1=OP.mult,
                    )
                    nc.vector.scalar_tensor_tensor(
                        out=t2,
                        in0=th_if[:, 2:4, :],
                        scalar=1.0,
                        in1=cd_f,
                        op0=OP.add,
                        op1=OP.mult,
                    )
                    # c'_new = 0.5 * t2 + p1
                    nc.vector.scalar_tensor_tensor(
                        out=cd_f,
                        in0=t2,
                        scalar=0.5,
                        in1=p1,
                        op0=OP.mult,
                        op1=OP.add,
                    )
                    th_c = wp.tile([128, 2, BL], dt.float32, tag="th_c")
                    nc.scalar.activation(
                        out=th_c, in_=cd_f, func=AF.Tanh, scale=0.5
                    )
                    # h' = (th_o + 1) * th_c
                    nc.vector.scalar_tensor_tensor(
                        out=hdT,
                        in0=th_go[:, 2:4, :],
                        scalar=1.0,
                        in1=th_c,
                        op0=OP.add,
                        op1=OP.mult,
                    )
                    nc.vector.tensor_copy(cd_b, cd_f)

                    if t == dec_steps - 1:
                        # dot_f ; out = hd.Wf + dot_f/Z + bf  (batch-major)
                        eq2 = wp.tile([128, TP], dt.bfloat16, tag="eq2")
                        dot_f = wp.tile([128, 1], dt.float32, tag="dot_f")
                        nc.vector.tensor_mul(eq2, e_sb, qfT[:, 0:TP])
                        nc.vector.tensor_reduce(
                            out=dot_f, in_=eq2, axis=mybir.AxisListType.X, op=OP.add
                        )
                        df = wp.tile([128, 1], dt.float32, tag="df")
                        nc.vector.tensor_mul(df, dot_f, rz)
                        fin_ps = ps_dx.tile([128, 1], dt.float32, tag="fin_ps")
                        for k in range(2):
                            nc.tensor.matmul(
                                fin_ps,
                                hdT[:, k, :],
                                sb["w_fh"][:, k, :],
                                start=(k == 0),
                                stop=(k == 1),
                            )
                        nc.vector.scalar_tensor_tensor(
                            out=o_col,
                            in0=df,
                            scalar=sb["consts_b"][:, 1:2],
                            in1=fin_ps,
                            op0=OP.add,
                            op1=OP.add,
                        )
                        nc.sync.dma_start(out=out_d.ap(), in_=o_col)

    nc.finalize()
    return nc


def _get_nc():
    if "nc" not in _CACHE:
        _CACHE["nc"] = _build()
    return _CACHE["nc"]


def _run(inputs, **kw):
    from concourse.bass_utils import run_bass_kernel_spmd

    shared, per_core = _pack_inputs(inputs)
    nc = _get_nc()
    in_maps = []
    for c in range(NCORES):
        m = dict(shared)
        m.update(per_core[c])
        in_maps.append(m)
    res = run_bass_kernel_spmd(nc, in_maps, list(range(NCORES)), **kw)
    out = np.concatenate([np.asarray(res.results[c]["out"]) for c in range(NCORES)])
    return out.astype(np.float32).reshape(B, 1), res


def kernel(**inputs):
    return _run(inputs)[0]


# revision 48
# speedup vs baseline: 1.4815x; 1.0125x over previous
"""DARNN (dual-attention RNN) Trainium2 Bass kernel — v2.

Strategy (pure data parallel, 8 cores, B=1024 -> 128 samples/core):

Activations are feature-major on-chip: x[b, f] lives in SBUF as xT[f, b]
(features on partitions, local batch on the free dim). Matmuls contract
over partitions with pre-transposed bf16 weights stationary.

v2 changes vs v1 baseline:
- Decoder softmax is batch-major: attn2 uses the (feature-major) tanh
  output as the *stationary* operand, producing scores [b, 257] in one
  PSUM bank (bias folded in via a ones-row matmul). A single Exp
  activation with accum_out yields e and Z; tensor_tensor_reduce yields
  dot_c; reciprocal runs at FD=1. xi returns to row layout via one PE
  transpose.
- All decoder sigmoids are rewritten as 0.5*tanh(x/2)+0.5 with the 0.5/2
  factors folded into packed weights (h'=2h, c'=2c representation), so
  the decoder only ever uses {Tanh, Exp} -> zero ACT table reloads.
- Decoder gate biases (+ comb_fc bias) are folded into a k=2 [xi; 1]
  matmul chunk, enabling two wide FD=512 gate activations.
- Fine-grained PSUM tiles (per attn1 m-chunk, per gate) let ACT start as
  soon as its slice is accumulated instead of after the whole block.
- PE issue order hides tanh/softmax latency behind h-only gate matmuls;
  the decoder attn1 feat contribution for step t+1 is prefilled into
  PSUM during step t's LSTM tail.
- Encoder hidden states live in one hs[128, 257, 2, BL] tile so the
  LSTM elementwise runs wide (FD=256) across both feature halves.
"""

import numpy as np
import ml_dtypes

B, T, NF, HE, HD = 1024, 256, 128, 256, 256
NCORES = 8
BL = B // NCORES  # 128 local batch
TP = T + 1  # 257
BF16 = ml_dtypes.bfloat16

_CACHE = {}


def _bf(x):
    return np.ascontiguousarray(np.asarray(x).astype(BF16))


def _f32(x):
    return np.ascontiguousarray(np.asarray(x).astype(np.float32))


def _pack_inputs(inputs):
    """Pack weights/biases into per-SBUF-tile layouts (shared across cores)."""
    f = {k: np.asarray(v, dtype=np.float32) for k, v in inputs.items()}
    p = {}

    # --- encoder attn1: a = tanh(cat[x,h,c] @ Wa1.T + ba1) ---
    # lhsT tile [128, 5, 257]: [p, k, m] = Wa1[m, k*128+p]
    wa1 = f["enc_Wa1"]  # [257, 640]
    p["w_enc_a1"] = _bf(wa1.T.reshape(5, 128, TP).transpose(1, 0, 2))
    b1 = np.zeros((128, 3), np.float32)
    b1.T.flat[:TP] = f["enc_ba1"]  # [p, j] = ba1[j*128+p]
    p["b_enc_a1"] = _f32(b1)

    # --- encoder attn2: alpha = a @ Wa2.T + ba2 ---  K=257 (3 chunks), M=128
    wa2 = f["enc_Wa2"]  # [128, 257]
    w = np.zeros((128, 3, 128), np.float32)
    w2 = wa2.T  # [257, 128]
    w[:, 0, :] = w2[0:128]
    w[:, 1, :] = w2[128:256]
    w[0, 2, :] = w2[256]
    p["w_enc_a2"] = _bf(w)
    p["b_enc_a2"] = _f32(f["enc_ba2"].reshape(128, 1))

    # --- encoder gates: g = Wih@xi + Whh@h + b ---  K chunks: [xi, h0, h1]
    # column order j = [i0,i1,f0,f1,g0,g1,o0,o1] (natural torch order)
    w = np.zeros((128, 3, 4 * HE), np.float32)
    w[:, 0, :] = f["enc_Wih"].T  # [128, 1024]
    w[:, 1, :] = f["enc_Whh"].T[0:128]
    w[:, 2, :] = f["enc_Whh"].T[128:256]
    p["w_enc_g"] = _bf(w)
    bsum = f["enc_bih"] + f["enc_bhh"]
    p["b_enc_g"] = _f32(bsum.reshape(8, 128).T)  # [p, j] = b[j*128+p]

    # --- q projections: q_c = h . Wc[0,1:], q_f = h . Wf[0,HD:] ---
    w = np.zeros((128, 2, 2), np.float32)
    w[:, 0, 0] = f["dec_Wc"][0, 1 : 1 + 128]
    w[:, 1, 0] = f["dec_Wc"][0, 129 : 1 + 256]
    w[:, 0, 1] = f["dec_Wf"][0, HD : HD + 128]
    w[:, 1, 1] = f["dec_Wf"][0, HD + 128 : HD + 256]
    p["w_q"] = _bf(w)

    # --- decoder attn1: a = tanh(cat[h,c,feat] @ Wa1.T + ba1) --- K=768 (6)
    # decoder h,c are stored as h'=2h, c'=2c -> scale those k-chunks by 0.5
    wa1d = f["dec_Wa1"].copy()  # [256, 768]
    wa1d[:, 0:512] *= 0.5  # h and c columns
    p["w_dec_a1"] = _bf(wa1d.T.reshape(6, 128, HE).transpose(1, 0, 2))
    p["b_dec_a1"] = _f32(f["dec_ba1"].reshape(2, 128).T)

    # --- decoder attn2 (batch-major): s[b, t'] = a.T @ Wa2dT + ba2 ---
    # moving operand [p=feat chunk, k, n=t'] = Wa2d[n, k*128+p]
    wa2d = f["dec_Wa2"]  # [257, 256]
    p["w_dec_a2"] = _bf(wa2d.T.reshape(2, 128, TP).transpose(1, 0, 2))
    ba2r = np.zeros((1, TP), np.float32)
    ba2r[0, :] = f["dec_ba2"]
    p["ba2_row"] = _bf(ba2r)

    # --- decoder gates ---
    # tanh-form LSTM: i,f,o gates become tanh(0.5*(pre+b)); g stays tanh.
    # Fold: h' = 2h -> Whh columns *0.5 ; g-gate pre-act scaled *2 so a
    # uniform ACT scale=0.5 works for the whole tile.
    sgate = np.ones((4 * HD,), np.float32)
    sgate[512:768] = 2.0  # g-gate columns
    whh = f["dec_Whh"].T * 0.5  # [256, 1024] (h' fold)
    w = np.zeros((128, 2, 4 * HD), np.float32)
    w[:, 0, :] = whh[0:128] * sgate
    w[:, 1, :] = whh[128:256] * sgate
    p["w_dec_g"] = _bf(w)
    # k=2 chunk [xi_nc; 1] with xi_nc = y*Wc00 + dot_c/Z  (bc folded here)
    wih = f["dec_Wih"][:, 0]  # [1024]
    bsum = f["dec_bih"] + f["dec_bhh"] + wih * f["dec_bc"][0]
    gx = np.zeros((2, 4 * HD), np.float32)
    gx[0, :] = wih * sgate
    gx[1, :] = bsum * sgate
    p["w_dec_gx"] = _bf(gx)

    # --- final: out = hd . Wf[0,:HD] + dot_f/Z + bf ---  (hd' = 2hd fold)
    w = np.zeros((128, 2, 1), np.float32)
    w[:, 0, 0] = f["dec_Wf"][0, 0:128] * 0.5
    w[:, 1, 0] = f["dec_Wf"][0, 128:256] * 0.5
    p["w_fh"] = _bf(w)

    # --- broadcast consts: [bc, bf, Wc00, 0] replicated over partitions ---
    cb = np.zeros((128, 4), np.float32)
    cb[:, 0] = f["dec_bc"][0]
    cb[:, 1] = f["dec_bf"][0]
    cb[:, 2] = f["dec_Wc"][0, 0]
    p["consts_b"] = _f32(cb)

    # --- identity for PE transposes ---
    p["ident"] = _bf(np.eye(128, dtype=np.float32))

    # --- per-core tensors ---
    feat = f["feat"]  # [B, 257, 128]
    target = f["target"]  # [B, 256]
    per_core = []
    for c in range(NCORES):
        sl = slice(c * BL, (c + 1) * BL)
        featT = _bf(feat[sl].transpose(2, 1, 0))  # [f=128, t=257, b=128]
        per_core.append({"featT": featT, "targetB": _f32(target[sl])})
    return p, per_core


def _build(enc_steps=TP, dec_steps=T):
    import concourse.mybir as mybir
    from concourse import bacc
    from concourse.tile import TileContext

    dt = mybir.dt
    AF = mybir.ActivationFunctionType
    OP = mybir.AluOpType

    nc = bacc.Bacc("TRN2")

    # ---- DRAM parameters ----
    dram = {}

    def din(name, shape, dtype):
        dram[name] = nc.declare_dram_parameter(name, list(shape), dtype, isOutput=False)

    din("featT", (128, TP, BL), dt.bfloat16)
    din("targetB", (BL, T), dt.float32)
    din("w_enc_a1", (128, 5, TP), dt.bfloat16)
    din("b_enc_a1", (128, 3), dt.float32)
    din("w_enc_a2", (128, 3, 128), dt.bfloat16)
    din("b_enc_a2", (128, 1), dt.float32)
    din("w_enc_g", (128, 3, 4 * HE), dt.bfloat16)
    din("b_enc_g", (128, 8), dt.float32)
    din("w_q", (128, 2, 2), dt.bfloat16)
    din("w_dec_a1", (128, 6, HE), dt.bfloat16)
    din("b_dec_a1", (128, 2), dt.float32)
    din("w_dec_a2", (128, 2, TP), dt.bfloat16)
    din("ba2_row", (1, TP), dt.bfloat16)
    din("w_dec_g", (128, 2, 4 * HD), dt.bfloat16)
    din("w_dec_gx", (2, 4 * HD), dt.bfloat16)
    din("w_fh", (128, 2, 1), dt.bfloat16)
    din("consts_b", (128, 4), dt.float32)
    din("ident", (128, 128), dt.bfloat16)
    out_d = nc.declare_dram_parameter("out", [BL], dt.float32, isOutput=True)

    with TileContext(nc) as tc:
        with (
            tc.tile_pool(name="consts", bufs=1) as cp,
            tc.tile_pool(name="state", bufs=1) as sp,
            tc.tile_pool(name="feat", bufs=8) as fp,
            tc.tile_pool(name="work", bufs=2) as wp,
        ):
            # ---- load weights into SBUF ----
            sb = {}
            for name, shape, dty in [
                ("w_enc_a1", (128, 5, TP), dt.bfloat16),
                ("b_enc_a1", (128, 3), dt.float32),
                ("w_enc_a2", (128, 3, 128), dt.bfloat16),
                ("b_enc_a2", (128, 1), dt.float32),
                ("w_enc_g", (128, 3, 4 * HE), dt.bfloat16),
                ("b_enc_g", (128, 8), dt.float32),
                ("w_q", (128, 2, 2), dt.bfloat16),
                ("w_dec_a1", (128, 6, HE), dt.bfloat16),
                ("b_dec_a1", (128, 2), dt.float32),
                ("w_dec_a2", (128, 2, TP), dt.bfloat16),
                ("ba2_row", (1, TP), dt.bfloat16),
                ("w_dec_g", (128, 2, 4 * HD), dt.bfloat16),
                ("w_dec_gx", (2, 4 * HD), dt.bfloat16),
                ("w_fh", (128, 2, 1), dt.bfloat16),
                ("consts_b", (128, 4), dt.float32),
                ("ident", (128, 128), dt.bfloat16),
                ("targetB", (BL, T), dt.float32),
            ]:
                t = cp.tile(list(shape), dty, tag=name)
                nc.sync.dma_start(out=t, in_=dram[name].ap())
                sb[name] = t

            ones_row = cp.tile([1, 128], dt.bfloat16, tag="ones_row")
            nc.vector.memset(ones_row, 1.0)
            zero_bf = cp.tile([128, BL], dt.bfloat16, tag="zero")
            nc.vector.memset(zero_bf, 0.0)

            # persistent big buffers
            hs = cp.tile([128, TP, 2, BL], dt.bfloat16, tag="hs")  # [f, t, half, b]
            qT = cp.tile([128, 3, 2, BL], dt.bfloat16, tag="qT")  # [t'%128, t'//128, {c,f}, b]
            nc.vector.memset(qT, 0.0)
            qcT = cp.tile([128, 384], dt.bfloat16, tag="qcT")  # [b, t'] (padded)
            qfT = cp.tile([128, 384], dt.bfloat16, tag="qfT")

            # encoder state
            c_f = sp.tile([128, 2, BL], dt.float32, tag="c_f")
            c_b = sp.tile([128, 2, BL], dt.bfloat16, tag="c_b")
            nc.vector.memset(c_f, 0.0)
            nc.vector.memset(c_b, 0.0)

            # ================= encoder =================
            with (
                tc.tile_pool(name="ps_a1", bufs=2, space="PSUM") as ps_a1,
                tc.tile_pool(name="ps_g", bufs=2, space="PSUM") as ps_g,
                tc.tile_pool(name="ps_q", bufs=1, space="PSUM") as ps_q,
                tc.tile_pool(name="ps_fl", bufs=1, space="PSUM") as ps_fl,
            ):
                enxt = {}
                fts = {}

                def enc_prefill(t):
                    # x-part of attn1 for step t, off the critical chain
                    ft = fp.tile([128, BL], dt.bfloat16, tag="ft", name="ft")
                    nc.sync.dma_start(out=ft, in_=dram["featT"].ap()[:, t, :])
                    fts[t] = ft
                    a1 = ps_a1.tile([128, 4, BL], dt.float32, tag="a1", name="a1")
                    # stop must ride on a full-128-partition matmul (psum
                    # group state is per-partition), so emit m order 0,2,1
                    for i, (m, mm) in enumerate(((0, 128), (2, 1), (1, 128))):
                        nc.tensor.matmul(
                            a1[:mm, m, :],
                            sb["w_enc_a1"][:, 0, m * 128 : m * 128 + mm],
                            ft,
                            start=(i == 0),
                            stop=(i == 2),
                        )
                    enxt[t] = a1

                enc_prefill(0)

                for t in range(enc_steps):
                    ft = fts.pop(t)
                    a1 = enxt.pop(t)

                    if t == 0:
                        hp0, hp1 = zero_bf, zero_bf
                    else:
                        hp0, hp1 = hs[:, t - 1, 0, :], hs[:, t - 1, 1, :]
                    rhs_a1 = [ft, hp0, hp1, c_b[:, 0, :], c_b[:, 1, :]]
                    rhs_g = [None, hp0, hp1]

                    # attn1: aT [257 -> (128,128,1), b] ; one bank [m0,m1,m2,al]
                    a1m = [a1[:, 0, :], a1[:, 1, :], a1[:1, 2, :]]
                    for m, mm in enumerate((128, 128, 1)):
                        for k in range(1, 5):
                            nc.tensor.matmul(
                                a1m[m],
                                sb["w_enc_a1"][:, k, m * 128 : m * 128 + mm],
                                rhs_a1[k],
                                start=False,
                                stop=True,
                                skip_group_check=True,
                            )

                    # gates h-part early: fills PE while ACT does tanh
                    g_if = ps_g.tile([128, 4, BL], dt.float32, tag="g_if", name="g_if")
                    g_go = ps_g.tile([128, 4, BL], dt.float32, tag="g_go", name="g_go")
                    gsl = [g_if[:, 0:2, :], g_if[:, 2:4, :], g_go[:, 0:2, :], g_go[:, 2:4, :]]
                    # h-part: ONE group per bank spanning its 4 slots, so the
                    # late xi accumulates (below) see has_written bits intact
                    for g in range(4):
                        for half in range(2):
                            for k in (1, 2):
                                j = g * 2 + half
                                first = (g % 2, half, k) == (0, 0, 1)
                                last = (g % 2, half, k) == (1, 1, 2)
                                nc.tensor.matmul(
                                    gsl[g][:, half, :],
                                    sb["w_enc_g"][:, k, j * 128 : (j + 1) * 128],
                                    rhs_g[k],
                                    start=first,
                                    stop=last,
                                )

                    # tanh (ACT) per m-chunk
                    aT = [
                        wp.tile([128, BL], dt.bfloat16, tag="aT0", name="aT0"),
                        wp.tile([128, BL], dt.bfloat16, tag="aT1", name="aT1"),
                        wp.tile([1, BL], dt.bfloat16, tag="aT2", name="aT2"),
                    ]
                    for m, mm in enumerate((128, 128, 1)):
                        nc.scalar.activation(
                            out=aT[m][:mm, :],
                            in_=a1m[m][:mm, :],
                            func=AF.Tanh,
                            bias=sb["b_enc_a1"][:mm, m : m + 1],
                        )

                    # attn2 + xi = (alpha + ba2) * x_t
                    al_ps = a1[:, 3, :]
                    for k, kk in enumerate((128, 128, 1)):
                        nc.tensor.matmul(
                            al_ps,
                            sb["w_enc_a2"][:kk, k, :],
                            aT[k][:kk, :],
                            start=(k == 0),
                            stop=(k == 2),
                        )
                    xiT = wp.tile([128, BL], dt.bfloat16, tag="xiT")
                    nc.vector.scalar_tensor_tensor(
                        out=xiT,
                        in0=al_ps,
                        scalar=sb["b_enc_a2"][:, 0:1],
                        in1=ft,
                        op0=OP.add,
                        op1=OP.mult,
                    )

                    # gates xi-part: late accumulate into the closed groups
                    # (has_written bits already set -> HW accumulates; the
                    # group check is sim-only bookkeeping)
                    for g in range(4):
                        for half in range(2):
                            j = g * 2 + half
                            nc.tensor.matmul(
                                gsl[g][:, half, :],
                                sb["w_enc_g"][:, 0, j * 128 : (j + 1) * 128],
                                xiT,
                                start=False,
                                stop=True,
                                skip_group_check=True,
                            )

                    # LSTM elementwise; per-(gate,half) ACT starts as soon as
                    # that gate's psum closes, wide (FD=256) VEC ops after.
                    bg = sb["b_enc_g"]
                    sg = [
                        wp.tile([128, 2, BL], dt.float32, tag=f"s{n}", name=f"s{n}")
                        for n in "ifgo"
                    ]
                    for g, fn in ((0, AF.Sigmoid), (1, AF.Sigmoid), (2, AF.Tanh)):
                        for half in range(2):
                            j = g * 2 + half
                            nc.scalar.activation(
                                out=sg[g][:, half, :],
                                in_=gsl[g][:, half, :],
                                func=fn,
                                bias=bg[:, j : j + 1],
                            )
                    p1 = wp.tile([128, 2, BL], dt.float32, tag="p1")
                    t2 = wp.tile([128, 2, BL], dt.float32, tag="t2")
                    nc.vector.tensor_mul(p1, sg[0], sg[2])
                    nc.vector.tensor_mul(t2, sg[1], c_f)
                    nc.vector.tensor_add(c_f, t2, p1)
                    nc.vector.tensor_copy(c_b, c_f)
                    for half in range(2):
                        j = 6 + half
                        nc.scalar.activation(
                            out=sg[3][:, half, :],
                            in_=gsl[3][:, half, :],
                            func=AF.Sigmoid,
                            bias=bg[:, j : j + 1],
                        )
                    tc_ = wp.tile([128, 2, BL], dt.float32, tag="tc")
                    nc.scalar.activation(out=tc_, in_=c_f, func=AF.Tanh)
                    nc.vector.tensor_mul(hs[:, t, :, :], sg[3], tc_)

                    # prefill next step's attn1 x-part, then HAM-warmth
                    # fillers that stream during the LSTM tail's PE idle
                    if t + 1 < enc_steps:
                        enc_prefill(t + 1)
                    flt = ps_fl.tile([128, 512], dt.float32, tag="fl", name="fl")
                    for _ in range(5):
                        nc.tensor.matmul(
                            flt, sb["ident"], sb["w_enc_g"][:, 0, 0:512],
                            start=True, stop=True,
                        )

                    # q rows: q_{c,f}[t] = h_t . W -> [2, b] -> DMA to qT row
                    q_ps = ps_q.tile([2, BL], dt.float32, tag="q_ps")
                    for k in range(2):
                        nc.tensor.matmul(
                            q_ps,
                            sb["w_q"][:, k, :],
                            hs[:, t, k, :],
                            start=(k == 0),
                            stop=(k == 1),
                        )
                    q_row = fp.tile([2, BL], dt.bfloat16, tag="q_row")
                    nc.vector.tensor_copy(q_row, q_ps)
                    nc.sync.dma_start(
                        out=qT[t % 128 : t % 128 + 1, t // 128, :, :], in_=q_row
                    )

            # ======= transpose q to batch-major qcT/qfT [b, t'] =======
            with tc.tile_pool(name="ps_tr", bufs=2, space="PSUM") as ps_tr:
                for chunk in range(3):
                    for cf in range(2):
                        tp_ps = ps_tr.tile([128, 128], dt.bfloat16, tag="tp")
                        nc.tensor.transpose(
                            tp_ps, qT[:, chunk, cf, :], sb["ident"]
                        )
                        dst = qcT if cf == 0 else qfT
                        nc.vector.tensor_copy(
                            dst[:, chunk * 128 : (chunk + 1) * 128], tp_ps
                        )

            # ================= decoder =================
            hdT = sp.tile([128, 2, BL], dt.bfloat16, tag="hdT")  # h' = 2h
            cd_f = sp.tile([128, 2, BL], dt.float32, tag="cd_f")  # c' = 2c
            cd_b = sp.tile([128, 2, BL], dt.bfloat16, tag="cd_b")
            nc.vector.memset(hdT, 0.0)
            nc.vector.memset(cd_f, 0.0)
            nc.vector.memset(cd_b, 0.0)
            xiT2 = sp.tile([2, BL], dt.bfloat16, tag="xiT2")  # [xi_nc; 1]
            nc.vector.memset(xiT2, 1.0)  # row 1 stays ones
            o_col = sp.tile([128, 1], dt.float32, tag="o_col")

            with (
                tc.tile_pool(name="ps_da1", bufs=2, space="PSUM") as ps_da1,
                tc.tile_pool(name="ps_s", bufs=1, space="PSUM") as ps_s,
                tc.tile_pool(name="ps_dg", bufs=1, space="PSUM") as ps_dg,
                tc.tile_pool(name="ps_dx", bufs=1, space="PSUM") as ps_dx,
                tc.tile_pool(name="ps_fl2", bufs=1, space="PSUM") as ps_fl2,
            ):
                nxt = {}

                def prefill(t):
                    da1 = ps_da1.tile([128, 2, BL], dt.float32, tag="da1", name="da1")
                    for m in range(2):
                        for k in range(2):  # feat chunks (4, 5)
                            nc.tensor.matmul(
                                da1[:, m, :],
                                sb["w_dec_a1"][:, 4 + k, m * 128 : (m + 1) * 128],
                                hs[:, t, k, :],
                                start=(m == 0 and k == 0),
                                stop=(m == 1 and k == 1),
                            )
                    nxt[t] = da1

                prefill(0)

                for t in range(dec_steps):
                    da1 = nxt.pop(t)
                    rhs_a1 = [
                        hdT[:, 0, :],
                        hdT[:, 1, :],
                        cd_b[:, 0, :],
                        cd_b[:, 1, :],
                    ]
                    for m in range(2):
                        for k in range(4):
                            nc.tensor.matmul(
                                da1[:, m, :],
                                sb["w_dec_a1"][:, k, m * 128 : (m + 1) * 128],
                                rhs_a1[k],
                                start=False,
                                stop=True,
                                skip_group_check=True,
                            )

                    # gates h-part early (only needs hdT)
                    g_if = ps_dg.tile([128, 4, BL], dt.float32, tag="g_if")
                    g_go = ps_dg.tile([128, 4, BL], dt.float32, tag="g_go")
                    for tile_, j0 in ((g_if, 0), (g_go, 4)):
                        for jl in range(4):
                            j = j0 + jl
                            for k in range(2):
                                nc.tensor.matmul(
                                    tile_[:, jl, :],
                                    sb["w_dec_g"][:, k, j * 128 : (j + 1) * 128],
                                    hdT[:, k, :],
                                    start=(jl == 0 and k == 0),
                                    stop=(jl == 3 and k == 1),
                                )

                    # attn1 tanh
                    aTd = wp.tile([128, 2, BL], dt.bfloat16, tag="aTd")
                    for m in range(2):
                        nc.scalar.activation(
                            out=aTd[:, m, :],
                            in_=da1[:, m, :],
                            func=AF.Tanh,
                            bias=sb["b_dec_a1"][:, m : m + 1],
                        )

                    # attn2, batch-major: s[b, t'] ; bias via ones-row matmul
                    s_ps = ps_s.tile(
                        [128, TP], dt.float32, tag="s_ps", padded_shape=[128, 512]
                    )
                    nc.tensor.matmul(
                        s_ps, ones_row, sb["ba2_row"], start=True, stop=False
                    )
                    for k in range(2):
                        nc.tensor.matmul(
                            s_ps,
                            aTd[:, k, :],
                            sb["w_dec_a2"][:, k, :],
                            start=False,
                            stop=(k == 1),
                        )

                    # softmax pieces: e, Z, dot_c, xi
                    e_sb = wp.tile([128, TP], dt.bfloat16, tag="e_sb")
                    z_t = wp.tile([128, 1], dt.float32, tag="z_t")
                    nc.scalar.activation(
                        out=e_sb, in_=s_ps, func=AF.Exp, accum_out=z_t
                    )
                    rz = wp.tile([128, 1], dt.float32, tag="rz")
                    nc.vector.reciprocal(rz, z_t)
                    eq = wp.tile([128, TP], dt.bfloat16, tag="eq")
                    dot_c = wp.tile([128, 1], dt.float32, tag="dot_c")
                    nc.vector.tensor_mul(eq, e_sb, qcT[:, 0:TP])
                    nc.vector.tensor_reduce(
                        out=dot_c, in_=eq, axis=mybir.AxisListType.X, op=OP.add
                    )
                    dcz = wp.tile([128, 1], dt.float32, tag="dcz")
                    nc.vector.tensor_mul(dcz, dot_c, rz)
                    xi_col = wp.tile([128, 1], dt.bfloat16, tag="xi_col")
                    nc.vector.scalar_tensor_tensor(
                        out=xi_col,
                        in0=sb["targetB"][:, t : t + 1],
                        scalar=sb["consts_b"][:, 2:3],
                        in1=dcz,
                        op0=OP.mult,
                        op1=OP.add,
                    )
                    # xi back to row layout: regular matmul against identity
                    # (out[0, n] = sum_k xi[k] * I[k, n] = xi[n])
                    xi_ps = ps_dx.tile([1, 128], dt.float32, tag="xi_ps")
                    nc.tensor.matmul(
                        xi_ps, xi_col, sb["ident"], start=True, stop=True
                    )
                    nc.vector.tensor_copy(xiT2[0:1, :], xi_ps[0:1, :])

                    # gates [xi; 1] chunk (bias folded): late accumulate
                    for tile_, j0 in ((g_if, 0), (g_go, 4)):
                        for jl in range(4):
                            j = j0 + jl
                            nc.tensor.matmul(
                                tile_[:, jl, :],
                                sb["w_dec_gx"][:, j * 128 : (j + 1) * 128],
                                xiT2,
                                start=False,
                                stop=True,
                                skip_group_check=True,
                            )

                    # prefill next step's attn1 feat part during the tail,
                    # plus HAM-warmth fillers in the PE-idle tail window
                    if t + 1 < dec_steps:
                        prefill(t + 1)
                    flt = ps_fl2.tile([128, 512], dt.float32, tag="fl2", name="fl2")
                    for _ in range(6):
                        nc.tensor.matmul(
                            flt, sb["ident"], sb["w_dec_g"][:, 0, 0:512],
                            start=True, stop=True,
                        )

                    # LSTM elementwise (tanh-form), wide ops
                    th_if = wp.tile([128, 4, BL], dt.float32, tag="th_if")
                    th_go = wp.tile([128, 4, BL], dt.float32, tag="th_go")
                    nc.scalar.activation(
                        out=th_if, in_=g_if, func=AF.Tanh, scale=0.5
                    )
                    nc.scalar.activation(
                        out=th_go, in_=g_go, func=AF.Tanh, scale=0.5
                    )
                    p1 = wp.tile([128, 2, BL], dt.float32, tag="dp1")
                    t2 = wp.tile([128, 2, BL], dt.float32, tag="dt2")
                    # t2 first: it only needs th_if, which finishes before th_go
                    nc.vector.scalar_tensor_tensor(
                        out=t2,
                        in0=th_if[:, 2:4, :],
                        scalar=1.0,
                        in1=cd_f,
                        op0=OP.add,
                        op1=OP.mult,
                    )
                    nc.vector.scalar_tensor_tensor(
                        out=p1,
                        in0=th_if[:, 0:2, :],
                        scalar=1.0,
                        in1=th_go[:, 0:2, :],
                        op0=OP.add,
                        op1=OP.mult,
                    )
                    # c'_new = 0.5 * t2 + p1
                    nc.vector.scalar_tensor_tensor(
                        out=cd_f,
                        in0=t2,
                        scalar=0.5,
                        in1=p1,
                        op0=OP.mult,
                        op1=OP.add,
                    )
                    th_c = wp.tile([128, 2, BL], dt.float32, tag="th_c")
                    nc.scalar.activation(
                        out=th_c, in_=cd_f, func=AF.Tanh, scale=0.5
                    )
                    # h' = (th_o + 1) * th_c
                    nc.vector.scalar_tensor_tensor(
                        out=hdT,
                        in0=th_go[:, 2:4, :],
                        scalar=1.0,
                        in1=th_c,
                        op0=OP.add,
                        op1=OP.mult,
                    )
                    nc.vector.tensor_copy(cd_b, cd_f)

                    if t == dec_steps - 1:
                        # dot_f ; out = hd.Wf + dot_f/Z + bf  (batch-major)
                        eq2 = wp.tile([128, TP], dt.bfloat16, tag="eq2")
                        dot_f = wp.tile([128, 1], dt.float32, tag="dot_f")
                        nc.vector.tensor_mul(eq2, e_sb, qfT[:, 0:TP])
                        nc.vector.tensor_reduce(
                            out=dot_f, in_=eq2, axis=mybir.AxisListType.X, op=OP.add
                        )
                        df = wp.tile([128, 1], dt.float32, tag="df")
                        nc.vector.tensor_mul(df, dot_f, rz)
                        fin_ps = ps_dx.tile([128, 1], dt.float32, tag="fin_ps")
                        for k in range(2):
                            nc.tensor.matmul(
                                fin_ps,
                                hdT[:, k, :],
                                sb["w_fh"][:, k, :],
                                start=(k == 0),
                                stop=(k == 1),
                            )
                        nc.vector.scalar_tensor_tensor(
                            out=o_col,
                            in0=df,
                            scalar=sb["consts_b"][:, 1:2],
                            in1=fin_ps,
                            op0=OP.add,
                            op1=OP.add,
                        )
                        nc.sync.dma_start(out=out_d.ap(), in_=o_col)

    nc.finalize()
    return nc


def _get_nc():
    if "nc" not in _CACHE:
        _CACHE["nc"] = _build()
    return _CACHE["nc"]


def _run(inputs, **kw):
    from concourse.bass_utils import run_bass_kernel_spmd

    shared, per_core = _pack_inputs(inputs)
    nc = _get_nc()
    in_maps = []
    for c in range(NCORES):
        m = dict(shared)
        m.update(per_core[c])
        in_maps.append(m)
    res = run_bass_kernel_spmd(nc, in_maps, list(range(NCORES)), **kw)
    out = np.concatenate([np.asarray(res.results[c]["out"]) for c in range(NCORES)])
    return out.astype(np.float32).reshape(B, 1), res


def kernel(**inputs):
    return _run(inputs)[0]
